# revision 6
# baseline (speedup 1.0000x reference)
"""Trainium2 Bass kernel for nn_Net_34729105555716.

Model: embedding lookup [30000,100] -> input projection (w_ih) -> 200-step
tanh RNN (hidden 300) -> relu MLP (300->256->3) over batch 4096.

Strategy (data-parallel over batch, 512 rows per core, 8 cores):
  - The fp16 embedding table lives in SBUF in dma_gather's
    tokens_per_rank=128 layout (partition = token%128, rank = token//128,
    rows padded 100->128 elems = 256B).
  - Per RNN step, a transpose-mode SWDGE dma_gather pulls the 512 tokens'
    embedding rows directly into matmul rhs layout [emb_dim(partitions),
    batch(free)], batched 2 steps per gather instruction.
  - PE does 12 fp16 matmuls per step (3 input-projection + 9 recurrent,
    K-tiles 128/128/44, M-tiles 128/128/44, N=512) accumulating in PSUM.
  - ScalarE applies tanh with the per-hidden bias (b_ih+b_hh) fused,
    writing the fp16 hidden state for the next step.
  - MLP head: 6 + 2 matmuls, relu fused on ScalarE, fc2 bias on VectorE.
Host side only marshals inputs (dtype cast, transpose, index layout) and
transposes the [3,512] per-core outputs back to [4096,3].
"""

import sys

if "/opt/trn_rl_repo" not in sys.path:
    sys.path.insert(0, "/opt/trn_rl_repo")

import numpy as np

SEQ = 200
BATCH = 4096
VOCAB = 30000
EMB = 100
HID = 300
FC1 = 256
N_CORES = 8
BPC = BATCH // N_CORES  # batch per core
G = 1  # RNN steps per dma_gather instruction (>512 idxs per gather crashes)
N_RANKS = (VOCAB + 127) // 128  # 235
KT = [(0, 128), (128, 128), (256, 44)]  # hidden-dim tiles

_cached = {}


def _split_multiwait(nc, mybir):
    """walrus in this container rejects >1 embedded sync wait per
    instruction (>2 for EventSemaphore); split extras onto NoOp carriers."""
    n = 0
    for f in nc.m.functions:
        for blk in f.blocks:
            if not any(
                i.sync_info is not None and len(i.sync_info.on_wait) > 1
                for i in blk.instructions
            ):
                continue
            out = []
            for inst in blk.instructions:
                si = inst.sync_info
                cap = 2 if isinstance(inst, mybir.InstEventSemaphore) else 1
                if si is not None and len(si.on_wait) > cap:
                    waits = list(si.on_wait)
                    for w in waits[:-cap]:
                        n += 1
                        carrier = mybir.InstNoOp(
                            name=f"I-waitsplit-{n}", ins=[], outs=[]
                        )
                        carrier.engine = inst.engine
                        carrier.sync_info = mybir.SyncInfo(
                            on_wait=[w], on_update=[]
                        )
                        out.append(carrier)
                    si.on_wait = waits[-cap:]
                out.append(inst)
            blk.instructions = out
    return n


def _build(seq=SEQ):
    import concourse.bass as bass
    import concourse.mybir as mybir
    import concourse.tile as tile
    from concourse import library_config
    from concourse.tile import add_dep_helper

    dt = mybir.dt
    f16, f32, i16 = dt.float16, dt.float32, dt.int16
    Tanh = mybir.ActivationFunctionType.Tanh
    Relu = mybir.ActivationFunctionType.Relu

    nc = bass.Bass(
        "TRN2",
        target_bir_lowering=False,
        debug=False,
        num_devices=N_CORES,
        num_swdge_queues=2,
    )
    x_idx = nc.dram_tensor(
        "x_idx", [128, seq * BPC // 16], i16, kind="ExternalInput"
    )
    emb_sb = nc.dram_tensor(
        "emb_sb", [128, N_RANKS * 128], f16, kind="ExternalInput"
    )
    whh_t = nc.dram_tensor("whh_t", [HID, HID], f16, kind="ExternalInput")
    wih_t = nc.dram_tensor("wih_t", [EMB, HID], f16, kind="ExternalInput")
    fc1_t = nc.dram_tensor("fc1_t", [HID, FC1], f16, kind="ExternalInput")
    fc2_t = nc.dram_tensor("fc2_t", [FC1, 3], f16, kind="ExternalInput")
    bias_sb = nc.dram_tensor("bias_sb", [128, 3], f32, kind="ExternalInput")
    fc1b_sb = nc.dram_tensor("fc1b_sb", [128, 2], f32, kind="ExternalInput")
    fc2b_sb = nc.dram_tensor("fc2b_sb", [3, 1], f32, kind="ExternalInput")
    out = nc.dram_tensor("out", [3, BPC], f32, kind="ExternalOutput")

    with tile.TileContext(nc) as tc:
        with (
            tc.tile_pool(name="const", bufs=1) as cpool,
            tc.tile_pool(name="gather", bufs=8) as gpool,
            tc.tile_pool(name="h", bufs=2) as hpool,
            tc.tile_pool(name="psum", bufs=2, space="PSUM") as ppool,
        ):
            lib_inst = nc.gpsimd.load_library(library_config.mlp)

            tbl = cpool.tile([128, N_RANKS * 128], f16, tag="tbl")
            nc.sync.dma_start(tbl[:], emb_sb.ap())
            idx = cpool.tile([128, seq * BPC // 16], i16, tag="idx")
            nc.sync.dma_start(idx[:], x_idx.ap())

            whh = []
            for o, sz in KT:
                w = cpool.tile([sz, HID], f16, tag=f"whh{o}")
                nc.sync.dma_start(w[:], whh_t.ap()[o : o + sz, :])
                whh.append(w)
            wih = cpool.tile([EMB, HID], f16, tag="wih")
            nc.sync.dma_start(wih[:], wih_t.ap())
            fc1 = []
            for o, sz in KT:
                w = cpool.tile([sz, FC1], f16, tag=f"fc1{o}")
                nc.sync.dma_start(w[:], fc1_t.ap()[o : o + sz, :])
                fc1.append(w)
            fc2 = []
            for o in (0, 128):
                w = cpool.tile([128, 3], f16, tag=f"fc2{o}")
                nc.sync.dma_start(w[:], fc2_t.ap()[o : o + 128, :])
                fc2.append(w)
            bias_t = cpool.tile([128, 3], f32, tag="bias")
            nc.sync.dma_start(bias_t[:], bias_sb.ap())
            fc1b_t = cpool.tile([128, 2], f32, tag="fc1b")
            nc.sync.dma_start(fc1b_t[:], fc1b_sb.ap())
            fc2b_t = cpool.tile([3, 1], f32, tag="fc2b")
            nc.sync.dma_start(fc2b_t[:], fc2b_sb.ap())

            reg_n = nc.gpsimd.to_reg(G * BPC)

            h = []
            for mi, (o, sz) in enumerate(KT):
                ht = hpool.tile([sz, BPC], f16, tag=f"h{mi}")
                nc.vector.memset(ht[:], 0)
                h.append(ht)

            xg = None
            for t in range(seq):
                g, sl = divmod(t, G)
                if sl == 0:
                    n_idx = min(G, seq - t) * BPC
                    xg = gpool.tile([128, 1, G * BPC], f16, tag="xe")
                    gi = nc.gpsimd.dma_gather(
                        xg[:, :, :n_idx],
                        tbl[:],
                        idx[:, g * (G * BPC // 16) : g * (G * BPC // 16) + n_idx // 16],
                        n_idx,
                        reg_n,
                        128,
                        transpose=True,
                        sbuf_tokens_per_rank=128,
                        sbuf_free_dim_per_rank=256,
                        queue_num=g % 2,
                    )
                    add_dep_helper(
                        gi.ins, lib_inst.ins, sync=False, reason="lib first"
                    )
                xe = xg[0:EMB, 0, sl * BPC : (sl + 1) * BPC]
                # K-contiguous per M-tile: recurrent MMs first (depend only on
                # h), the gather-dependent input projection last (stop=True),
                # so each psum tile completes early and tanh overlaps the
                # remaining M-tiles' matmuls.
                hn = []
                for mi, (mo, ms) in enumerate(KT):
                    p = ppool.tile([ms, BPC], f32, tag=f"p{mi}")
                    for ki, (ko, ks) in enumerate(KT):
                        nc.tensor.matmul(
                            p[:],
                            whh[ki][:, mo : mo + ms],
                            h[ki][:],
                            start=(ki == 0),
                            stop=False,
                        )
                    nc.tensor.matmul(
                        p[:], wih[:, mo : mo + ms], xe, start=False, stop=True
                    )
                    ht = hpool.tile([ms, BPC], f16, tag=f"h{mi}")
                    nc.scalar.activation(
                        ht[:], p[:], Tanh, bias=bias_t[0:ms, mi : mi + 1]
                    )
                    hn.append(ht)
                h = hn

            # MLP head
            h1 = []
            for mi in range(2):
                p = ppool.tile([128, BPC], f32, tag=f"p{mi}")
                for ki, (ko, ks) in enumerate(KT):
                    nc.tensor.matmul(
                        p[:],
                        fc1[ki][:, mi * 128 : (mi + 1) * 128],
                        h[ki][:],
                        start=(ki == 0),
                        stop=(ki == len(KT) - 1),
                    )
                ht = hpool.tile([128, BPC], f16, tag=f"h1_{mi}")
                nc.scalar.activation(
                    ht[:], p[:], Relu, bias=fc1b_t[:, mi : mi + 1]
                )
                h1.append(ht)
            p2 = ppool.tile([3, BPC], f32, tag="p2")
            nc.tensor.matmul(p2[:], fc2[0][:, :], h1[0][:], start=True, stop=False)
            nc.tensor.matmul(p2[:], fc2[1][:, :], h1[1][:], start=False, stop=True)
            osb = cpool.tile([3, BPC], f32, tag="osb")
            nc.vector.tensor_scalar_add(osb[:], p2[:], fc2b_t[:, 0:1])
            nc.sync.dma_start(out.ap(), osb[:])

    # Populate .instr bytes for InstISA subclasses (library reload etc.) —
    # Bacc.compile does this; raw Bass+Tile must do it explicitly or walrus
    # fails with "ISA wrong length".
    mybir.codegen_inst_isa_subclasses(nc)
    _split_multiwait(nc, mybir)
    return nc


def _prep_inputs(x, emb, w_ih, w_hh, b_ih, b_hh, fc1_w, fc1_b, fc2_w, fc2_b,
                 seq=SEQ):
    """Marshal the model inputs into per-core DRAM input maps."""
    x = np.asarray(x)
    assert x.shape == (seq, BATCH), x.shape

    # Embedding table in SBUF-gather layout: partition = token % 128,
    # rank = token // 128, 128 fp16 elems (256B) per row.
    emb_pad = np.zeros((N_RANKS * 128, 128), np.float16)
    emb_pad[:VOCAB, :EMB] = np.asarray(emb, np.float16)
    emb_sb = np.ascontiguousarray(
        emb_pad.reshape(N_RANKS, 128, 128).transpose(1, 0, 2).reshape(128, -1)
    )

    whh_t = np.ascontiguousarray(np.asarray(w_hh, np.float16).T)  # [in, out]
    wih_t = np.ascontiguousarray(np.asarray(w_ih, np.float16).T)  # [emb, hid]
    fc1_t = np.ascontiguousarray(np.asarray(fc1_w, np.float16).T)  # [hid, 256]
    fc2_t = np.ascontiguousarray(np.asarray(fc2_w, np.float16).T)  # [256, 3]

    bias = np.zeros(384, np.float32)
    bias[:HID] = np.asarray(b_ih, np.float32) + np.asarray(b_hh, np.float32)
    bias_sb = np.ascontiguousarray(bias.reshape(3, 128).T)  # [128, 3]
    fc1b_sb = np.ascontiguousarray(
        np.asarray(fc1_b, np.float32).reshape(2, 128).T
    )
    fc2b_sb = np.asarray(fc2_b, np.float32).reshape(3, 1)

    shared = {
        "emb_sb": emb_sb,
        "whh_t": whh_t,
        "wih_t": wih_t,
        "fc1_t": fc1_t,
        "fc2_t": fc2_t,
        "bias_sb": bias_sb,
        "fc1b_sb": fc1b_sb,
        "fc2b_sb": fc2b_sb,
    }
    in_maps = []
    for c in range(N_CORES):
        xc = x[:, c * BPC : (c + 1) * BPC]  # [seq, 512]
        flat = np.ascontiguousarray(xc).reshape(-1).astype(np.int16)
        block = np.ascontiguousarray(flat.reshape(-1, 16).T)  # [16, seq*BPC/16]
        x_idx = np.ascontiguousarray(np.tile(block, (8, 1)))  # [128, ...]
        in_maps.append({"x_idx": x_idx, **shared})
    return in_maps


def _get_nc():
    if "nc" not in _cached:
        _cached["nc"] = _build()
    return _cached["nc"]


def _assemble(results):
    # per-core out is [3, 512]; assemble full [4096, 3]
    full = np.concatenate([r["out"].T for r in results], axis=0)
    return full.astype(np.float32)


def kernel(x, emb, w_ih, w_hh, b_ih, b_hh, fc1_w, fc1_b, fc2_w, fc2_b):
    from concourse.bass_utils import run_bass_kernel_spmd

    nc = _get_nc()
    in_maps = _prep_inputs(
        x, emb, w_ih, w_hh, b_ih, b_hh, fc1_w, fc1_b, fc2_w, fc2_b
    )
    res = run_bass_kernel_spmd(nc, in_maps, core_ids=list(range(N_CORES)))
    return _assemble(res.results)



# revision 7
# speedup vs baseline: 1.2189x; 1.2189x over previous
"""Trainium2 Bass kernel for nn_Net_34729105555716.

Model: embedding lookup [30000,100] -> input projection (w_ih) -> 200-step
tanh RNN (hidden 300) -> relu MLP (300->256->3) over batch 4096.

Strategy (data-parallel over batch, 512 rows per core, 8 cores):
  - The input projection + bias is FOLDED INTO THE EMBEDDING TABLE on the
    host: proj_table[v] = emb[v] @ w_ih.T + (b_ih + b_hh), stored fp16 in
    SBUF in dma_gather transpose layout (partition = token%128, rank =
    token//128, rows padded 300->384 elems = 768B).
  - Per RNN step, ONE transpose-mode SWDGE dma_gather (elem_size=384)
    pulls the 512 tokens' projected rows directly into [128, 3, 512] --
    rank r holds hidden dims 128r+p, exactly the 3 M-tile PSUM layout.
  - PE does only the 9 recurrent fp16 matmuls per step (K-tiles
    128/128/44 contiguous per M-tile), accumulating in PSUM.
  - VectorE adds the gathered xp tile into PSUM (scalar_tensor_tensor),
    ScalarE applies tanh (bias already folded), writing fp16 h.
  - MLP head: 6 + 2 matmuls, relu fused on ScalarE, fc2 bias on VectorE.
Host side only marshals inputs (weight-only refactor, dtype cast,
transpose, index layout) and transposes the [3,512] per-core outputs back
to [4096,3].
"""

import sys

if "/opt/trn_rl_repo" not in sys.path:
    sys.path.insert(0, "/opt/trn_rl_repo")

import numpy as np

SEQ = 200
BATCH = 4096
VOCAB = 30000
EMB = 100
HID = 300
FC1 = 256
N_CORES = 8
BPC = BATCH // N_CORES  # batch per core
N_RANKS = (VOCAB + 127) // 128  # 235
ROW_ELEMS = 384  # projected row padded 300 -> 384 fp16 elems (768 B)
KT = [(0, 128), (128, 128), (256, 44)]  # hidden-dim tiles
IDX_CHUNK = 40  # steps of indices per SBUF-resident chunk

_cached = {}


def _split_multiwait(nc, mybir):
    """walrus in this container rejects >1 embedded sync wait per
    instruction (>2 for EventSemaphore); split extras onto NoOp carriers."""
    n = 0
    for f in nc.m.functions:
        for blk in f.blocks:
            if not any(
                i.sync_info is not None and len(i.sync_info.on_wait) > 1
                for i in blk.instructions
            ):
                continue
            out = []
            for inst in blk.instructions:
                si = inst.sync_info
                cap = 2 if isinstance(inst, mybir.InstEventSemaphore) else 1
                if si is not None and len(si.on_wait) > cap:
                    waits = list(si.on_wait)
                    for w in waits[:-cap]:
                        n += 1
                        carrier = mybir.InstNoOp(
                            name=f"I-waitsplit-{n}", ins=[], outs=[]
                        )
                        carrier.engine = inst.engine
                        carrier.sync_info = mybir.SyncInfo(
                            on_wait=[w], on_update=[]
                        )
                        out.append(carrier)
                    si.on_wait = waits[-cap:]
                out.append(inst)
            blk.instructions = out
    return n


def _build(seq=SEQ):
    import concourse.bass as bass
    import concourse.mybir as mybir
    import concourse.tile as tile
    from concourse import library_config
    from concourse.tile import add_dep_helper

    dt = mybir.dt
    f16, f32, i16 = dt.float16, dt.float32, dt.int16
    Tanh = mybir.ActivationFunctionType.Tanh
    Relu = mybir.ActivationFunctionType.Relu
    Mult = mybir.AluOpType.mult
    Add = mybir.AluOpType.add

    nc = bass.Bass(
        "TRN2",
        target_bir_lowering=False,
        debug=False,
        num_devices=N_CORES,
        num_swdge_queues=2,
    )
    x_idx = nc.dram_tensor(
        "x_idx", [128, seq * BPC // 16], i16, kind="ExternalInput"
    )
    emb_sb = nc.dram_tensor(
        "emb_sb", [128, N_RANKS * ROW_ELEMS], f16, kind="ExternalInput"
    )
    whh_t = nc.dram_tensor("whh_t", [HID, HID], f16, kind="ExternalInput")
    fc1_t = nc.dram_tensor("fc1_t", [HID, FC1], f16, kind="ExternalInput")
    fc2_t = nc.dram_tensor("fc2_t", [FC1, 3], f16, kind="ExternalInput")
    fc1b_sb = nc.dram_tensor("fc1b_sb", [128, 2], f32, kind="ExternalInput")
    fc2b_sb = nc.dram_tensor("fc2b_sb", [3, 1], f32, kind="ExternalInput")
    out = nc.dram_tensor("out", [3, BPC], f32, kind="ExternalOutput")

    idx_cols = IDX_CHUNK * BPC // 16  # idx i16 columns per chunk

    with tile.TileContext(nc) as tc:
        with (
            tc.tile_pool(name="const", bufs=1) as cpool,
            tc.tile_pool(name="idxp", bufs=2) as ipool,
            tc.tile_pool(name="gather", bufs=3) as gpool,
            tc.tile_pool(name="h", bufs=2) as hpool,
            tc.tile_pool(name="psum", bufs=2, space="PSUM") as ppool,
        ):
            lib_inst = nc.gpsimd.load_library(library_config.mlp)

            tbl = cpool.tile([128, N_RANKS * ROW_ELEMS], f16, tag="tbl")
            nc.sync.dma_start(tbl[:], emb_sb.ap())

            whh = []
            for o, sz in KT:
                w = cpool.tile([sz, HID], f16, tag=f"whh{o}")
                nc.sync.dma_start(w[:], whh_t.ap()[o : o + sz, :])
                whh.append(w)
            fc1 = []
            for o, sz in KT:
                w = cpool.tile([sz, FC1], f16, tag=f"fc1{o}")
                nc.sync.dma_start(w[:], fc1_t.ap()[o : o + sz, :])
                fc1.append(w)
            fc2 = []
            for o in (0, 128):
                w = cpool.tile([128, 3], f16, tag=f"fc2{o}")
                nc.sync.dma_start(w[:], fc2_t.ap()[o : o + 128, :])
                fc2.append(w)
            fc1b_t = cpool.tile([128, 2], f32, tag="fc1b")
            nc.sync.dma_start(fc1b_t[:], fc1b_sb.ap())
            fc2b_t = cpool.tile([3, 1], f32, tag="fc2b")
            nc.sync.dma_start(fc2b_t[:], fc2b_sb.ap())

            reg_n = nc.gpsimd.to_reg(BPC)

            h = []
            for mi, (o, sz) in enumerate(KT):
                ht = hpool.tile([sz, BPC], f16, tag=f"h{mi}")
                nc.vector.memset(ht[:], 0)
                h.append(ht)

            idx_t = None
            for t in range(seq):
                c, sl = divmod(t, IDX_CHUNK)
                if sl == 0:
                    idx_t = ipool.tile([128, idx_cols], i16, tag="idx")
                    nc.sync.dma_start(
                        idx_t[:],
                        x_idx.ap()[:, c * idx_cols : (c + 1) * idx_cols],
                    )
                xg = gpool.tile([128, HID // 128 + 1, BPC], f16, tag="xe")
                gi = nc.gpsimd.dma_gather(
                    xg[:],
                    tbl[:],
                    idx_t[:, sl * (BPC // 16) : (sl + 1) * (BPC // 16)],
                    BPC,
                    reg_n,
                    ROW_ELEMS,
                    transpose=True,
                    sbuf_tokens_per_rank=128,
                    sbuf_free_dim_per_rank=ROW_ELEMS * 2,
                    queue_num=t % 2,
                )
                if t == 0:
                    add_dep_helper(
                        gi.ins, lib_inst.ins, sync=False, reason="lib first"
                    )
                # K-contiguous per M-tile: psum completes early so the
                # DVE add + tanh of tile mi overlap the remaining tiles'
                # matmuls.
                hn = []
                for mi, (mo, ms) in enumerate(KT):
                    p = ppool.tile([ms, BPC], f32, tag=f"p{mi}")
                    for ki, (ko, ks) in enumerate(KT):
                        nc.tensor.matmul(
                            p[:],
                            whh[ki][:, mo : mo + ms],
                            h[ki][:],
                            start=(ki == 0),
                            stop=(ki == len(KT) - 1),
                        )
                    # p += xp (gathered projected embedding, bias folded)
                    nc.vector.scalar_tensor_tensor(
                        p[:], xg[0:ms, mi, :], 1.0, p[:], Mult, Add
                    )
                    ht = hpool.tile([ms, BPC], f16, tag=f"h{mi}")
                    nc.scalar.activation(ht[:], p[:], Tanh)
                    hn.append(ht)
                h = hn

            # MLP head
            h1 = []
            for mi in range(2):
                p = ppool.tile([128, BPC], f32, tag=f"p{mi}")
                for ki, (ko, ks) in enumerate(KT):
                    nc.tensor.matmul(
                        p[:],
                        fc1[ki][:, mi * 128 : (mi + 1) * 128],
                        h[ki][:],
                        start=(ki == 0),
                        stop=(ki == len(KT) - 1),
                    )
                ht = hpool.tile([128, BPC], f16, tag=f"h1_{mi}")
                nc.scalar.activation(
                    ht[:], p[:], Relu, bias=fc1b_t[:, mi : mi + 1]
                )
                h1.append(ht)
            p2 = ppool.tile([3, BPC], f32, tag="p2")
            nc.tensor.matmul(p2[:], fc2[0][:, :], h1[0][:], start=True, stop=False)
            nc.tensor.matmul(p2[:], fc2[1][:, :], h1[1][:], start=False, stop=True)
            osb = cpool.tile([3, BPC], f32, tag="osb")
            nc.vector.tensor_scalar_add(osb[:], p2[:], fc2b_t[:, 0:1])
            nc.sync.dma_start(out.ap(), osb[:])

    # Populate .instr bytes for InstISA subclasses (library reload etc.) —
    # Bacc.compile does this; raw Bass+Tile must do it explicitly or walrus
    # fails with "ISA wrong length".
    mybir.codegen_inst_isa_subclasses(nc)
    _split_multiwait(nc, mybir)
    return nc


def _prep_inputs(x, emb, w_ih, w_hh, b_ih, b_hh, fc1_w, fc1_b, fc2_w, fc2_b,
                 seq=SEQ):
    """Marshal the model inputs into per-core DRAM input maps."""
    x = np.asarray(x)
    assert x.shape == (seq, BATCH), x.shape

    # Weight-only refactor: fold the input projection + bias into the
    # table: proj_table[v] = emb[v] @ w_ih.T + (b_ih + b_hh).
    proj = (
        np.asarray(emb, np.float32) @ np.asarray(w_ih, np.float32).T
        + np.asarray(b_ih, np.float32)
        + np.asarray(b_hh, np.float32)
    ).astype(np.float16)  # [VOCAB, HID]

    # SBUF-gather layout: partition = token % 128, rank = token // 128,
    # rows padded 300 -> 384 fp16 elems (768 B).
    emb_pad = np.zeros((N_RANKS * 128, ROW_ELEMS), np.float16)
    emb_pad[:VOCAB, :HID] = proj
    emb_sb = np.ascontiguousarray(
        emb_pad.reshape(N_RANKS, 128, ROW_ELEMS)
        .transpose(1, 0, 2)
        .reshape(128, -1)
    )

    whh_t = np.ascontiguousarray(np.asarray(w_hh, np.float16).T)  # [in, out]
    fc1_t = np.ascontiguousarray(np.asarray(fc1_w, np.float16).T)  # [hid, 256]
    fc2_t = np.ascontiguousarray(np.asarray(fc2_w, np.float16).T)  # [256, 3]

    fc1b_sb = np.ascontiguousarray(
        np.asarray(fc1_b, np.float32).reshape(2, 128).T
    )
    fc2b_sb = np.asarray(fc2_b, np.float32).reshape(3, 1)

    shared = {
        "emb_sb": emb_sb,
        "whh_t": whh_t,
        "fc1_t": fc1_t,
        "fc2_t": fc2_t,
        "fc1b_sb": fc1b_sb,
        "fc2b_sb": fc2b_sb,
    }
    in_maps = []
    for c in range(N_CORES):
        xc = x[:, c * BPC : (c + 1) * BPC]  # [seq, 512]
        flat = np.ascontiguousarray(xc).reshape(-1).astype(np.int16)
        block = np.ascontiguousarray(flat.reshape(-1, 16).T)  # [16, seq*BPC/16]
        x_idx = np.ascontiguousarray(np.tile(block, (8, 1)))  # [128, ...]
        in_maps.append({"x_idx": x_idx, **shared})
    return in_maps


def _get_nc():
    if "nc" not in _cached:
        _cached["nc"] = _build()
    return _cached["nc"]


def _assemble(results):
    # per-core out is [3, 512]; assemble full [4096, 3]
    full = np.concatenate([r["out"].T for r in results], axis=0)
    return full.astype(np.float32)


def kernel(x, emb, w_ih, w_hh, b_ih, b_hh, fc1_w, fc1_b, fc2_w, fc2_b):
    from concourse.bass_utils import run_bass_kernel_spmd

    nc = _get_nc()
    in_maps = _prep_inputs(
        x, emb, w_ih, w_hh, b_ih, b_hh, fc1_w, fc1_b, fc2_w, fc2_b
    )
    res = run_bass_kernel_spmd(nc, in_maps, core_ids=list(range(N_CORES)))
    return _assemble(res.results)


# revision 16
# speedup vs baseline: 1.2260x; 1.0058x over previous
"""Host-xp variant: the embedding lookup + input projection for all steps
is precomputed on the host; the kernel streams xp tiles from HBM with
plain HWDGE DMAs (no SWDGE gathers at all) and runs the 9 recurrent
matmuls + DVE add + tanh per step.
"""

import sys

if "/opt/trn_rl_repo" not in sys.path:
    sys.path.insert(0, "/opt/trn_rl_repo")

import numpy as np

SEQ = 200
BATCH = 4096
VOCAB = 30000
EMB = 100
HID = 300
FC1 = 256
N_CORES = 8
BPC = BATCH // N_CORES
KT = [(0, 128), (128, 128), (256, 44)]
XP_BUFS = 6

_cached = {}


def _split_multiwait(nc, mybir):
    n = 0
    for f in nc.m.functions:
        for blk in f.blocks:
            if not any(
                i.sync_info is not None and len(i.sync_info.on_wait) > 1
                for i in blk.instructions
            ):
                continue
            out = []
            for inst in blk.instructions:
                si = inst.sync_info
                cap = 2 if isinstance(inst, mybir.InstEventSemaphore) else 1
                if si is not None and len(si.on_wait) > cap:
                    waits = list(si.on_wait)
                    for w in waits[:-cap]:
                        n += 1
                        carrier = mybir.InstNoOp(
                            name=f"I-waitsplit-{n}", ins=[], outs=[]
                        )
                        carrier.engine = inst.engine
                        carrier.sync_info = mybir.SyncInfo(
                            on_wait=[w], on_update=[]
                        )
                        out.append(carrier)
                    si.on_wait = waits[-cap:]
                out.append(inst)
            blk.instructions = out
    return n


def _build(seq=SEQ):
    import concourse.bass as bass
    import concourse.mybir as mybir
    import concourse.tile as tile

    dt = mybir.dt
    f16, f32 = dt.float16, dt.float32
    Tanh = mybir.ActivationFunctionType.Tanh
    Relu = mybir.ActivationFunctionType.Relu
    Mult = mybir.AluOpType.mult
    Add = mybir.AluOpType.add

    nc = bass.Bass(
        "TRN2", target_bir_lowering=False, debug=False, num_devices=N_CORES
    )
    STEP_F = 3 * BPC  # fp16 elems per step per partition
    xp_all = nc.dram_tensor(
        "xp_all", [128, seq * STEP_F], f16, kind="ExternalInput"
    )
    whh_t = nc.dram_tensor("whh_t", [HID, HID], f16, kind="ExternalInput")
    fc1_t = nc.dram_tensor("fc1_t", [HID, FC1], f16, kind="ExternalInput")
    fc2_t = nc.dram_tensor("fc2_t", [FC1, 3], f16, kind="ExternalInput")
    fc1b_sb = nc.dram_tensor("fc1b_sb", [128, 2], f32, kind="ExternalInput")
    fc2b_sb = nc.dram_tensor("fc2b_sb", [3, 1], f32, kind="ExternalInput")
    out = nc.dram_tensor("out", [3, BPC], f32, kind="ExternalOutput")

    with tile.TileContext(nc) as tc:
        with (
            tc.tile_pool(name="const", bufs=1) as cpool,
            tc.tile_pool(name="xp", bufs=XP_BUFS) as xpool,
            tc.tile_pool(name="h", bufs=2) as hpool,
            tc.tile_pool(name="psum", bufs=2, space="PSUM") as ppool,
        ):
            whh = []
            for o, sz in KT:
                w = cpool.tile([sz, HID], f16, tag=f"whh{o}")
                nc.sync.dma_start(w[:], whh_t.ap()[o : o + sz, :])
                whh.append(w)
            fc1 = []
            for o, sz in KT:
                w = cpool.tile([sz, FC1], f16, tag=f"fc1{o}")
                nc.sync.dma_start(w[:], fc1_t.ap()[o : o + sz, :])
                fc1.append(w)
            fc2 = []
            for o in (0, 128):
                w = cpool.tile([128, 3], f16, tag=f"fc2{o}")
                nc.sync.dma_start(w[:], fc2_t.ap()[o : o + 128, :])
                fc2.append(w)
            fc1b_t = cpool.tile([128, 2], f32, tag="fc1b")
            nc.sync.dma_start(fc1b_t[:], fc1b_sb.ap())
            fc2b_t = cpool.tile([3, 1], f32, tag="fc2b")
            nc.sync.dma_start(fc2b_t[:], fc2b_sb.ap())

            h = []
            for mi, (o, sz) in enumerate(KT):
                ht = hpool.tile([sz, BPC], f16, tag=f"h{mi}")
                nc.vector.memset(ht[:], 0)
                h.append(ht)

            for t in range(seq):
                xg = xpool.tile([128, 3, BPC], f16, tag="xp")
                nc.sync.dma_start(
                    xg[:], xp_all.ap()[:, t * STEP_F : (t + 1) * STEP_F]
                )
                hn = []
                for mi, (mo, ms) in enumerate(KT):
                    p = ppool.tile([ms, BPC], f32, tag=f"p{mi}")
                    for ki, (ko, ks) in enumerate(KT):
                        nc.tensor.matmul(
                            p[:],
                            whh[ki][:, mo : mo + ms],
                            h[ki][:],
                            start=(ki == 0),
                            stop=(ki == len(KT) - 1),
                        )
                    nc.vector.scalar_tensor_tensor(
                        p[:], xg[0:ms, mi, :], 1.0, p[:], Mult, Add
                    )
                    ht = hpool.tile([ms, BPC], f16, tag=f"h{mi}")
                    nc.scalar.activation(ht[:], p[:], Tanh)
                    hn.append(ht)
                h = hn

            h1 = []
            for mi in range(2):
                p = ppool.tile([128, BPC], f32, tag=f"p{mi}")
                for ki, (ko, ks) in enumerate(KT):
                    nc.tensor.matmul(
                        p[:],
                        fc1[ki][:, mi * 128 : (mi + 1) * 128],
                        h[ki][:],
                        start=(ki == 0),
                        stop=(ki == len(KT) - 1),
                    )
                ht = hpool.tile([128, BPC], f16, tag=f"h1_{mi}")
                nc.scalar.activation(
                    ht[:], p[:], Relu, bias=fc1b_t[:, mi : mi + 1]
                )
                h1.append(ht)
            p2 = ppool.tile([3, BPC], f32, tag="p2")
            nc.tensor.matmul(p2[:], fc2[0][:, :], h1[0][:], start=True, stop=False)
            nc.tensor.matmul(p2[:], fc2[1][:, :], h1[1][:], start=False, stop=True)
            osb = cpool.tile([3, BPC], f32, tag="osb")
            nc.vector.tensor_scalar_add(osb[:], p2[:], fc2b_t[:, 0:1])
            nc.sync.dma_start(out.ap(), osb[:])

    mybir.codegen_inst_isa_subclasses(nc)
    _split_multiwait(nc, mybir)
    return nc


def _prep_inputs(x, emb, w_ih, w_hh, b_ih, b_hh, fc1_w, fc1_b, fc2_w, fc2_b,
                 seq=SEQ):
    x = np.asarray(x)
    assert x.shape == (seq, BATCH), x.shape

    proj = (
        np.asarray(emb, np.float32) @ np.asarray(w_ih, np.float32).T
        + np.asarray(b_ih, np.float32)
        + np.asarray(b_hh, np.float32)
    ).astype(np.float16)  # [VOCAB, HID]
    proj_pad = np.zeros((VOCAB, 384), np.float16)
    proj_pad[:, :HID] = proj

    whh_t = np.ascontiguousarray(np.asarray(w_hh, np.float16).T)
    fc1_t = np.ascontiguousarray(np.asarray(fc1_w, np.float16).T)
    fc2_t = np.ascontiguousarray(np.asarray(fc2_w, np.float16).T)
    fc1b_sb = np.ascontiguousarray(
        np.asarray(fc1_b, np.float32).reshape(2, 128).T
    )
    fc2b_sb = np.asarray(fc2_b, np.float32).reshape(3, 1)

    shared = {
        "whh_t": whh_t,
        "fc1_t": fc1_t,
        "fc2_t": fc2_t,
        "fc1b_sb": fc1b_sb,
        "fc2b_sb": fc2b_sb,
    }
    in_maps = []
    for c in range(N_CORES):
        xc = x[:, c * BPC : (c + 1) * BPC]  # [seq, 512]
        xp = proj_pad[xc]  # [seq, 512, 384]
        # -> [128 partition, seq, 3 rank, 512 batch]
        xp = xp.reshape(seq, BPC, 3, 128).transpose(3, 0, 2, 1)
        xp_all = np.ascontiguousarray(xp).reshape(128, -1)
        in_maps.append({"xp_all": xp_all, **shared})
    return in_maps


def _get_nc():
    if "nc" not in _cached:
        _cached["nc"] = _build()
    return _cached["nc"]


def _assemble(results):
    full = np.concatenate([r["out"].T for r in results], axis=0)
    return full.astype(np.float32)


def kernel(x, emb, w_ih, w_hh, b_ih, b_hh, fc1_w, fc1_b, fc2_w, fc2_b):
    from concourse.bass_utils import run_bass_kernel_spmd

    nc = _get_nc()
    in_maps = _prep_inputs(
        x, emb, w_ih, w_hh, b_ih, b_hh, fc1_w, fc1_b, fc2_w, fc2_b
    )
    res = run_bass_kernel_spmd(nc, in_maps, core_ids=list(range(N_CORES)))
    return _assemble(res.results)


# revision 19
# speedup vs baseline: 2.2020x; 1.7960x over previous
"""Trainium2 Bass kernel for nn_Net_34729105555716.

Model: embedding lookup [30000,100] -> input projection (w_ih) -> 200-step
tanh RNN (hidden 300) -> relu MLP (300->256->3) over batch 4096.

Strategy (data-parallel over batch, 512 rows per core, 8 cores):
  - The fp16 embedding table lives in SBUF in dma_gather's
    tokens_per_rank=128 layout (partition = token%128, rank = token//128,
    rows padded 100->128 elems = 256B).
  - Per RNN step, a transpose-mode SWDGE dma_gather pulls the 512 tokens'
    embedding rows directly into matmul rhs layout [emb_dim(partitions),
    batch(free)]. Gathers alternate between 2 SWDGE queues and prefetch
    8 steps deep, keeping them off the critical path (one 512-token
    gather costs ~4.6 us of Q7 time regardless of row bytes).
  - PE does 12 fp16 matmuls per step (3 input-projection + 9 recurrent,
    K-tiles 128/128/44, M-tiles 128/128/44, N=512) accumulating in PSUM.
  - ScalarE applies tanh with the per-hidden bias (b_ih+b_hh) fused,
    writing the fp16 hidden state for the next step.
  - MLP head: 6 + 2 matmuls, relu fused on ScalarE, fc2 bias on VectorE.
Host side only marshals inputs (dtype cast, transpose, index layout) and
transposes the [3,512] per-core outputs back to [4096,3].
"""

import sys

if "/opt/trn_rl_repo" not in sys.path:
    sys.path.insert(0, "/opt/trn_rl_repo")

import numpy as np

SEQ = 200
BATCH = 4096
VOCAB = 30000
EMB = 100
HID = 300
FC1 = 256
N_CORES = 8
BPC = BATCH // N_CORES  # batch per core
G = 1  # RNN steps per dma_gather instruction (>512 idxs per gather crashes)
N_RANKS = (VOCAB + 127) // 128  # 235
HIDP = 384  # hidden padded to 3 uniform 128 tiles (zeros)
KT = [(0, 128), (128, 128), (256, 128)]  # hidden-dim tiles

_cached = {}


def _split_multiwait(nc, mybir):
    """walrus in this container rejects >1 embedded sync wait per
    instruction (>2 for EventSemaphore); split extras onto NoOp carriers."""
    n = 0
    for f in nc.m.functions:
        for blk in f.blocks:
            if not any(
                i.sync_info is not None and len(i.sync_info.on_wait) > 1
                for i in blk.instructions
            ):
                continue
            out = []
            for inst in blk.instructions:
                si = inst.sync_info
                cap = 2 if isinstance(inst, mybir.InstEventSemaphore) else 1
                if si is not None and len(si.on_wait) > cap:
                    waits = list(si.on_wait)
                    for w in waits[:-cap]:
                        n += 1
                        carrier = mybir.InstNoOp(
                            name=f"I-waitsplit-{n}", ins=[], outs=[]
                        )
                        carrier.engine = inst.engine
                        carrier.sync_info = mybir.SyncInfo(
                            on_wait=[w], on_update=[]
                        )
                        out.append(carrier)
                    si.on_wait = waits[-cap:]
                out.append(inst)
            blk.instructions = out
    return n


def _build(seq=SEQ):
    import concourse.bass as bass
    import concourse.mybir as mybir
    import concourse.tile as tile
    from concourse import library_config
    from concourse.tile import add_dep_helper

    dt = mybir.dt
    f16, f32, i16 = dt.float16, dt.float32, dt.int16
    Tanh = mybir.ActivationFunctionType.Tanh
    Relu = mybir.ActivationFunctionType.Relu

    nc = bass.Bass(
        "TRN2",
        target_bir_lowering=False,
        debug=False,
        num_devices=N_CORES,
        num_swdge_queues=2,
    )
    x_idx = nc.dram_tensor(
        "x_idx", [128, seq * BPC // 16], i16, kind="ExternalInput"
    )
    emb_sb = nc.dram_tensor(
        "emb_sb", [128, N_RANKS * 128], f16, kind="ExternalInput"
    )
    whh_t = nc.dram_tensor("whh_t", [HIDP, HIDP], f16, kind="ExternalInput")
    wih_t = nc.dram_tensor("wih_t", [128, HIDP], f16, kind="ExternalInput")
    fc1_t = nc.dram_tensor("fc1_t", [HIDP, FC1], f16, kind="ExternalInput")
    fc2_t = nc.dram_tensor("fc2_t", [FC1, 3], f16, kind="ExternalInput")
    bias_sb = nc.dram_tensor("bias_sb", [128, 3], f32, kind="ExternalInput")
    fc1b_sb = nc.dram_tensor("fc1b_sb", [128, 2], f32, kind="ExternalInput")
    fc2b_sb = nc.dram_tensor("fc2b_sb", [3, 1], f32, kind="ExternalInput")
    out = nc.dram_tensor("out", [3, BPC], f32, kind="ExternalOutput")

    with tile.TileContext(nc) as tc:
        with (
            tc.tile_pool(name="const", bufs=1) as cpool,
            tc.tile_pool(name="gather", bufs=8) as gpool,
            tc.tile_pool(name="h", bufs=2) as hpool,
            tc.tile_pool(name="psum", bufs=2, space="PSUM") as ppool,
        ):
            lib_inst = nc.gpsimd.load_library(library_config.mlp)

            tbl = cpool.tile([128, N_RANKS * 128], f16, tag="tbl")
            nc.sync.dma_start(tbl[:], emb_sb.ap())
            idx = cpool.tile([128, seq * BPC // 16], i16, tag="idx")
            nc.sync.dma_start(idx[:], x_idx.ap())

            whh = []
            for o, sz in KT:
                w = cpool.tile([sz, HIDP], f16, tag=f"whh{o}")
                nc.sync.dma_start(w[:], whh_t.ap()[o : o + sz, :])
                whh.append(w)
            wih = cpool.tile([128, HIDP], f16, tag="wih")
            nc.sync.dma_start(wih[:], wih_t.ap())
            fc1 = []
            for o, sz in KT:
                w = cpool.tile([sz, FC1], f16, tag=f"fc1{o}")
                nc.sync.dma_start(w[:], fc1_t.ap()[o : o + sz, :])
                fc1.append(w)
            fc2 = []
            for o in (0, 128):
                w = cpool.tile([128, 3], f16, tag=f"fc2{o}")
                nc.sync.dma_start(w[:], fc2_t.ap()[o : o + 128, :])
                fc2.append(w)
            bias_t = cpool.tile([128, 3], f32, tag="bias")
            nc.sync.dma_start(bias_t[:], bias_sb.ap())
            fc1b_t = cpool.tile([128, 2], f32, tag="fc1b")
            nc.sync.dma_start(fc1b_t[:], fc1b_sb.ap())
            fc2b_t = cpool.tile([3, 1], f32, tag="fc2b")
            nc.sync.dma_start(fc2b_t[:], fc2b_sb.ap())

            reg_n = nc.gpsimd.to_reg(G * BPC)

            h = []
            for mi, (o, sz) in enumerate(KT):
                ht = hpool.tile([sz, BPC], f16, tag=f"h{mi}")
                nc.vector.memset(ht[:], 0)
                h.append(ht)

            xg = None
            for t in range(seq):
                g, sl = divmod(t, G)
                if sl == 0:
                    n_idx = min(G, seq - t) * BPC
                    xg = gpool.tile([128, 1, G * BPC], f16, tag="xe")
                    gi = nc.gpsimd.dma_gather(
                        xg[:, :, :n_idx],
                        tbl[:],
                        idx[:, g * (G * BPC // 16) : g * (G * BPC // 16) + n_idx // 16],
                        n_idx,
                        reg_n,
                        128,
                        transpose=True,
                        sbuf_tokens_per_rank=128,
                        sbuf_free_dim_per_rank=256,
                        queue_num=g % 2,
                    )
                    add_dep_helper(
                        gi.ins, lib_inst.ins, sync=False, reason="lib first"
                    )
                xe = xg[0:128, 0, sl * BPC : (sl + 1) * BPC]
                ps = []
                for mi, (o, sz) in enumerate(KT):
                    p = ppool.tile([sz, BPC], f32, tag=f"p{mi}")
                    nc.tensor.matmul(
                        p[:], wih[:, o : o + sz], xe, start=True, stop=False
                    )
                    ps.append(p)
                for ki, (ko, ks) in enumerate(KT):
                    last = ki == len(KT) - 1
                    for mi, (mo, ms) in enumerate(KT):
                        nc.tensor.matmul(
                            ps[mi][:],
                            whh[ki][:, mo : mo + ms],
                            h[ki][:],
                            start=False,
                            stop=last,
                        )
                hn = []
                for mi, (o, sz) in enumerate(KT):
                    ht = hpool.tile([sz, BPC], f16, tag=f"h{mi}")
                    nc.scalar.activation(
                        ht[:], ps[mi][:], Tanh, bias=bias_t[0:sz, mi : mi + 1]
                    )
                    hn.append(ht)
                h = hn

            # MLP head
            h1 = []
            for mi in range(2):
                p = ppool.tile([128, BPC], f32, tag=f"p{mi}")
                for ki, (ko, ks) in enumerate(KT):
                    nc.tensor.matmul(
                        p[:],
                        fc1[ki][:, mi * 128 : (mi + 1) * 128],
                        h[ki][:],
                        start=(ki == 0),
                        stop=(ki == len(KT) - 1),
                    )
                ht = hpool.tile([128, BPC], f16, tag=f"h1_{mi}")
                nc.scalar.activation(
                    ht[:], p[:], Relu, bias=fc1b_t[:, mi : mi + 1]
                )
                h1.append(ht)
            p2 = ppool.tile([3, BPC], f32, tag="p2")
            nc.tensor.matmul(p2[:], fc2[0][:, :], h1[0][:], start=True, stop=False)
            nc.tensor.matmul(p2[:], fc2[1][:, :], h1[1][:], start=False, stop=True)
            osb = cpool.tile([3, BPC], f32, tag="osb")
            nc.vector.tensor_scalar_add(osb[:], p2[:], fc2b_t[:, 0:1])
            nc.sync.dma_start(out.ap(), osb[:])

    # Populate .instr bytes for InstISA subclasses (library reload etc.) —
    # Bacc.compile does this; raw Bass+Tile must do it explicitly or walrus
    # fails with "ISA wrong length".
    mybir.codegen_inst_isa_subclasses(nc)
    _split_multiwait(nc, mybir)
    return nc


def _prep_inputs(x, emb, w_ih, w_hh, b_ih, b_hh, fc1_w, fc1_b, fc2_w, fc2_b,
                 seq=SEQ):
    """Marshal the model inputs into per-core DRAM input maps."""
    x = np.asarray(x)
    assert x.shape == (seq, BATCH), x.shape

    # Embedding table in SBUF-gather layout: partition = token % 128,
    # rank = token // 128, 128 fp16 elems (256B) per row.
    emb_pad = np.zeros((N_RANKS * 128, 128), np.float16)
    emb_pad[:VOCAB, :EMB] = np.asarray(emb, np.float16)
    emb_sb = np.ascontiguousarray(
        emb_pad.reshape(N_RANKS, 128, 128).transpose(1, 0, 2).reshape(128, -1)
    )

    whh_t = np.zeros((384, 384), np.float16)  # [in(pad), out(pad)]
    whh_t[:HID, :HID] = np.asarray(w_hh, np.float16).T
    wih_t = np.zeros((128, 384), np.float16)  # [emb(pad), hid(pad)]
    wih_t[:EMB, :HID] = np.asarray(w_ih, np.float16).T
    fc1_t = np.zeros((384, FC1), np.float16)  # [hid(pad), 256]
    fc1_t[:HID, :] = np.asarray(fc1_w, np.float16).T
    fc2_t = np.ascontiguousarray(np.asarray(fc2_w, np.float16).T)  # [256, 3]

    bias = np.zeros(384, np.float32)
    bias[:HID] = np.asarray(b_ih, np.float32) + np.asarray(b_hh, np.float32)
    bias_sb = np.ascontiguousarray(bias.reshape(3, 128).T)  # [128, 3]
    fc1b_sb = np.ascontiguousarray(
        np.asarray(fc1_b, np.float32).reshape(2, 128).T
    )
    fc2b_sb = np.asarray(fc2_b, np.float32).reshape(3, 1)

    shared = {
        "emb_sb": emb_sb,
        "whh_t": whh_t,
        "wih_t": wih_t,
        "fc1_t": fc1_t,
        "fc2_t": fc2_t,
        "bias_sb": bias_sb,
        "fc1b_sb": fc1b_sb,
        "fc2b_sb": fc2b_sb,
    }
    in_maps = []
    for c in range(N_CORES):
        xc = x[:, c * BPC : (c + 1) * BPC]  # [seq, 512]
        flat = np.ascontiguousarray(xc).reshape(-1).astype(np.int16)
        block = np.ascontiguousarray(flat.reshape(-1, 16).T)  # [16, seq*BPC/16]
        x_idx = np.ascontiguousarray(np.tile(block, (8, 1)))  # [128, ...]
        in_maps.append({"x_idx": x_idx, **shared})
    return in_maps


def _get_nc():
    if "nc" not in _cached:
        _cached["nc"] = _build()
    return _cached["nc"]


def _assemble(results):
    # per-core out is [3, 512]; assemble full [4096, 3]
    full = np.concatenate([r["out"].T for r in results], axis=0)
    return full.astype(np.float32)


def kernel(x, emb, w_ih, w_hh, b_ih, b_hh, fc1_w, fc1_b, fc2_w, fc2_b):
    from concourse.bass_utils import run_bass_kernel_spmd

    nc = _get_nc()
    in_maps = _prep_inputs(
        x, emb, w_ih, w_hh, b_ih, b_hh, fc1_w, fc1_b, fc2_w, fc2_b
    )
    res = run_bass_kernel_spmd(nc, in_maps, core_ids=list(range(N_CORES)))
    return _assemble(res.results)

